# revision 1
# baseline (speedup 1.0000x reference)
"""Trainium2 Bass kernel for nn_AttentionBlockManual (dense transformer block).

Reference computation (per batch element n):
    temb = relu(t @ W_t.T + b_t)                      # [C]
    xin  = x + temb[:, None, None]                    # [C, H, W]
    tokens: full spatial attention over L = H*W = 1024 tokens, dim C = 256
    q/k/v = proj(xin), scores = q k^T / 16, P = softmax, o = P v
    out  = o @ Wp.T + bp, transposed back, + residual x

Token relabeling note: the reference's transpose(1,3) is a pure permutation of
the 1024 tokens applied consistently to q/k/v and inverted on output; full
softmax attention is permutation-equivariant, so we use the natural memory
order (h-major) token index and skip both transposes.

Sharding: data-parallel over batch N=32 across 8 cores (4 batches per core),
params replicated. No collectives.

Layouts on chip (per batch):
    X    [C=256(part x2), L=1024]   fp32 (residual) ; Xin bf16
    Q, K [D=256(part x2), L=1024]   bf16
    V^T  [L=1024(part x8), D=256]   bf16
    S^T tiles [128 j, 512 i] in PSUM  (scores transposed; i = query tokens)
    expS^T -> bf16 SBUF (no max-subtraction needed: |s/16| <~ 6, exp is fp32)
    rowsums via an all-ones [128,128] matmul -- the result lands replicated
      across partitions, so 1/rowsum needs no cross-partition broadcast
    O^T  [D(part x2), 512 i] accumulated in PSUM over the 8 j tiles
    normalization (reciprocal_approx_fast) applied at the O^T PSUM->SBUF copy
    proj -> [C(part x2), 512 i] + bp + residual -> DRAM

Scheduling notes: the score loop is software-pipelined one j-tile deep,
carried across the query-half boundary, so each exp's latency hides under the
next score tile's matmuls (the PE queue is in-order);
DMA issue is split across the two HWDGE engines (sync/scalar) to use two
hardware queues, with x prefetched one batch ahead; the time embeddings for
all four batches are computed in one batched matmul at startup.
"""

from contextlib import ExitStack

import numpy as np

import concourse.bacc as bacc
import concourse.tile as tile
from concourse import mybir
from concourse.bass_utils import run_bass_kernel_spmd
from concourse.masks import make_identity

F32 = mybir.dt.float32
BF16 = mybir.dt.bfloat16
AF = mybir.ActivationFunctionType
ALU = mybir.AluOpType

N_CORES = 8
B = 4            # batches per core
C = 256          # channels
L = 1024         # tokens (H*W)
D = 256          # qk/v dim
T = 512          # time embedding dim
P = 128          # partitions
CT = C // P      # 2 channel chunks
DT = D // P      # 2 dim chunks
TT = T // P      # 4 time chunks
JT = L // P      # 8 key-token chunks
NH = 512         # moving-dim chunk (one PSUM bank of fp32)
IH = L // NH     # 2 query-token halves
SCALE = 1.0 / np.sqrt(256.0)


def _build_body(tc, x_d, t_d, wt_d, bt_d, wq_d, wk_d, wv_d, wp_d, bp_d, out_d):
    nc = tc.nc

    ctx = ExitStack()
    const = ctx.enter_context(tc.tile_pool(name="const", bufs=1))
    wraw = ctx.enter_context(tc.tile_pool(name="wraw", bufs=2))
    xpool = ctx.enter_context(tc.tile_pool(name="xp", bufs=2))
    xinp = ctx.enter_context(tc.tile_pool(name="xin", bufs=2))
    qpool = ctx.enter_context(tc.tile_pool(name="qp", bufs=2))
    kpool = ctx.enter_context(tc.tile_pool(name="kp", bufs=2))
    vpool = ctx.enter_context(tc.tile_pool(name="vp", bufs=2))
    espool = ctx.enter_context(tc.tile_pool(name="es", bufs=6))
    otnp = ctx.enter_context(tc.tile_pool(name="otn", bufs=2))
    rbp = ctx.enter_context(tc.tile_pool(name="rb", bufs=2))
    ypool = ctx.enter_context(tc.tile_pool(name="yp", bufs=4))
    # PSUM: shared 1-bank slots + the 2-bank O^T accumulators
    pss = ctx.enter_context(tc.tile_pool(name="pss", bufs=2, space="PSUM"))
    rsp = ctx.enter_context(tc.tile_pool(name="rsp", bufs=2, space="PSUM"))
    psot = ctx.enter_context(tc.tile_pool(name="psot", bufs=2, space="PSUM"))

    # ---- constants & weights -------------------------------------------
    ident = const.tile([P, P], F32, tag="ident")
    make_identity(nc, ident)
    ones_bf = const.tile([P, P], BF16, tag="ones")
    nc.vector.memset(ones_bf, 1.0)

    def load_transposed(w_d, n_in_chunks, n_free, name):
        """DRAM [rows, cols] -> SBUF bf16 [128, cols/128, rows] (transposed)."""
        n_out_chunks = n_free // P
        raw = wraw.tile([P, n_in_chunks, n_free], F32, tag="wraw")
        for a in range(n_in_chunks):
            nc.scalar.dma_start(out=raw[:, a, :], in_=w_d[a * P:(a + 1) * P, :])
        wT = const.tile([P, n_out_chunks, n_in_chunks * P], BF16, tag=name)
        for a in range(n_in_chunks):
            for b in range(n_out_chunks):
                ps = pss.tile([P, P], F32, tag="ps")
                nc.tensor.transpose(ps, raw[:, a, b * P:(b + 1) * P], ident)
                nc.scalar.copy(out=wT[:, b, a * P:(a + 1) * P], in_=ps)
        return wT

    x0_sb = xpool.tile([P, CT, L], F32, tag="x")
    for _ct in range(CT):
        nc.sync.dma_start(
            out=x0_sb[:, _ct, :],
            in_=x_d[0, _ct * P:(_ct + 1) * P, :, :].rearrange("c h w -> c (h w)"),
        )
    wtT = load_transposed(wt_d, CT, T, "wtT")   # [128t, 4(t), 256c]
    wqT = load_transposed(wq_d, CT, D, "wqT")   # [128c, 2(c), 256d]
    wkT = load_transposed(wk_d, CT, D, "wkT")
    wvT = load_transposed(wv_d, CT, D, "wvT")
    wpT = load_transposed(wp_d, DT, C, "wpT")   # [128d, 2(d), 256c]

    bp_sb = const.tile([P, CT], F32, tag="bp")
    bt_sb = const.tile([P, CT], F32, tag="bt")
    nc.sync.dma_start(out=bp_sb, in_=bp_d.rearrange("(ct p) -> p ct", p=P))
    nc.sync.dma_start(out=bt_sb, in_=bt_d.rearrange("(ct p) -> p ct", p=P))

    # time embeddings for all batches at once: temb_all[:, ct, n] = relu(W_t t_n + b_t)
    t_all = const.tile([P, TT, B], F32, tag="tall")
    for kt in range(TT):
        nc.sync.dma_start(out=t_all[:, kt, :],
                          in_=t_d[:, kt * P:(kt + 1) * P].rearrange("n p -> p n"))
    t_all_bf = const.tile([P, TT, B], BF16, tag="tallbf")
    nc.vector.tensor_copy(out=t_all_bf, in_=t_all)
    temb_all = const.tile([P, CT, B], F32, tag="temba")
    for ct in range(CT):
        tb_ps = pss.tile([P, B], F32, tag="ps")
        for kt in range(TT):
            nc.tensor.matmul(
                tb_ps, wtT[:, kt, ct * P:(ct + 1) * P], t_all_bf[:, kt, :],
                start=(kt == 0), stop=(kt == TT - 1),
            )
        nc.scalar.activation(
            out=temb_all[:, ct, :], in_=tb_ps, func=AF.Relu,
            bias=bt_sb[:, ct:ct + 1], scale=1.0,
        )

    # ---- per batch, software-pipelined emission -------------------------
    # Stage A(n): temb/xin/Q/K/V^T.  Stage B(n): score+PV loops.
    # Stage C(n): recip/normalize/project/residual tails.
    # Emitted as A(0), B(0), A(1), C(0), B(1), A(2), C(1), ... so the PE
    # always has independent matmul work while a tail's serial chain runs.
    st_state = {}

    x_tiles = {0: x0_sb}

    def load_x(n):
        if n >= B:
            return
        x_sb = xpool.tile([P, CT, L], F32, tag="x")
        for ct in range(CT):
            nc.sync.dma_start(
                out=x_sb[:, ct, :],
                in_=x_d[n, ct * P:(ct + 1) * P, :, :].rearrange("c h w -> c (h w)"),
            )
        x_tiles[n] = x_sb

    def emit_qkv(n):
        if n >= B:
            return
        load_x(n + 1)
        x_sb = x_tiles[n]

        # xin = x + temb  (bf16)
        xin = xinp.tile([P, CT, L], BF16, tag="xin")
        for ct in range(CT):
            nc.scalar.activation(out=xin[:, ct, :], in_=x_sb[:, ct, :],
                                 func=AF.Identity,
                                 bias=temb_all[:, ct, n:n + 1])

        # Q, K: [128d, 2(d), 1024 tok]
        q_bf = qpool.tile([P, DT, L], BF16, tag="q")
        k_bf = kpool.tile([P, DT, L], BF16, tag="k")
        for wT, dst in ((wqT, q_bf), (wkT, k_bf)):
            for m in range(DT):
                for nh in range(IH):
                    ps = pss.tile([P, NH], F32, tag="ps")
                    for kc in range(CT):
                        nc.tensor.matmul(
                            ps, wT[:, kc, m * P:(m + 1) * P],
                            xin[:, kc, nh * NH:(nh + 1) * NH],
                            start=(kc == 0), stop=(kc == CT - 1),
                        )
                    nc.vector.tensor_copy(out=dst[:, m, nh * NH:(nh + 1) * NH], in_=ps)

        # V^T: [128tok, 8(tok), 256d]
        vt_bf = vpool.tile([P, JT, D], BF16, tag="vt")
        for jt in range(JT):
            ps = pss.tile([P, D], F32, tag="ps")
            for kc in range(CT):
                nc.tensor.matmul(
                    ps, xin[:, kc, jt * P:(jt + 1) * P], wvT[:, kc, :],
                    start=(kc == 0), stop=(kc == CT - 1),
                )
            nc.vector.tensor_copy(out=vt_bf[:, jt, :], in_=ps)
        st_state[n] = dict(x_sb=x_sb, q_bf=q_bf, k_bf=k_bf, vt_bf=vt_bf)

    def emit_scores(n):
        s = st_state[n]
        q_bf, k_bf, vt_bf = s["q_bf"], s["k_bf"], s["vt_bf"]
        rs_list, ot_list = [], []
        for _ih in range(IH):
            rs_ps = rsp.tile([P, NH], F32, tag="rs")
            ot_ps = psot.tile([P, DT, NH], F32, tag="ot")
            rs_list.append(rs_ps)
            ot_list.append(ot_ps)

        # jt-pipelined one tile deep, carried ACROSS the query-half boundary:
        # rs/PV(jt) is emitted after the next score tile's matmuls so the
        # exp(jt) latency hides under them (the PE queue is in-order; a wait
        # on rs(jt) would stall the following S^T matmuls behind it).
        def emit_rs_pv(ih, jt, es):
            nc.tensor.matmul(rs_list[ih], ones_bf, es,
                             start=(jt == 0), stop=(jt == JT - 1))
            for dh in range(DT):
                nc.tensor.matmul(
                    ot_list[ih][:, dh, :], vt_bf[:, jt, dh * P:(dh + 1) * P],
                    es, start=(jt == 0), stop=(jt == JT - 1),
                )

        es_q = []
        for ih in range(IH):
            isl = slice(ih * NH, (ih + 1) * NH)
            for jt in range(JT):
                st_ps = pss.tile([P, NH], F32, tag="ps")
                for kc in range(DT):
                    nc.tensor.matmul(
                        st_ps, k_bf[:, kc, jt * P:(jt + 1) * P], q_bf[:, kc, isl],
                        start=(kc == 0), stop=(kc == DT - 1),
                    )
                es = espool.tile([P, NH], BF16, tag="es")
                nc.scalar.activation(out=es, in_=st_ps, func=AF.Exp, scale=SCALE)
                es_q.append((ih, jt, es))
                if len(es_q) > 1:
                    emit_rs_pv(*es_q.pop(0))
        emit_rs_pv(*es_q.pop(0))
        s["rs_list"], s["ot_list"] = rs_list, ot_list

    def emit_tails(n):
        s = st_state.pop(n)
        x_sb = s["x_sb"]
        otn_list = []
        for ih in range(IH):
            rs_ps, ot_ps = s["rs_list"][ih], s["ot_list"][ih]
            recip_b = rbp.tile([P, NH], F32, tag="recipb")
            nc.vector.reciprocal_approx_fast(out=recip_b, in_=rs_ps)
            otn = otnp.tile([P, DT, NH], BF16, tag="otn")
            for dh in range(DT):
                nc.vector.tensor_mul(otn[:, dh, :], ot_ps[:, dh, :], recip_b)
            otn_list.append(otn)
        for ih in range(IH):
            isl = slice(ih * NH, (ih + 1) * NH)
            otn = otn_list[ih]
            # projection + bias + residual
            for ct in range(CT):
                pj_ps = pss.tile([P, NH], F32, tag="ps")
                for dh in range(DT):
                    nc.tensor.matmul(
                        pj_ps, wpT[:, dh, ct * P:(ct + 1) * P], otn[:, dh, :],
                        start=(dh == 0), stop=(dh == DT - 1),
                    )
                y = ypool.tile([P, NH], F32, tag="y")
                nc.vector.scalar_tensor_tensor(
                    out=y, in0=pj_ps, scalar=bp_sb[:, ct:ct + 1],
                    in1=x_sb[:, ct, isl], op0=ALU.add, op1=ALU.add,
                )
                store_eng = nc.sync if (ih + ct) % 2 == 0 else nc.scalar
                store_eng.dma_start(
                    out=out_d[n, ct * P:(ct + 1) * P, :, :]
                    .rearrange("c h w -> c (h w)")[:, isl],
                    in_=y,
                )

    emit_qkv(0)
    for n in range(B):
        emit_scores(n)
        emit_tails(n)
        emit_qkv(n + 1)

    ctx.close()


_CACHE = {}


def _get_program():
    if "nc" in _CACHE:
        return _CACHE["nc"]
    nc = bacc.Bacc("TRN2", target_bir_lowering=False, debug=False,
                   num_devices=N_CORES)
    x_d = nc.dram_tensor("x", [B, C, 32, 32], F32, kind="ExternalInput").ap()
    t_d = nc.dram_tensor("t", [B, T], F32, kind="ExternalInput").ap()
    wt_d = nc.dram_tensor("W_t", [C, T], F32, kind="ExternalInput").ap()
    bt_d = nc.dram_tensor("b_t", [C], F32, kind="ExternalInput").ap()
    wq_d = nc.dram_tensor("Wq", [D, C], F32, kind="ExternalInput").ap()
    wk_d = nc.dram_tensor("Wk", [D, C], F32, kind="ExternalInput").ap()
    wv_d = nc.dram_tensor("Wv", [D, C], F32, kind="ExternalInput").ap()
    wp_d = nc.dram_tensor("Wp", [C, D], F32, kind="ExternalInput").ap()
    bp_d = nc.dram_tensor("bp", [C], F32, kind="ExternalInput").ap()
    out_d = nc.dram_tensor("out", [B, C, 32, 32], F32, kind="ExternalOutput").ap()

    with tile.TileContext(nc) as tc:
        _build_body(tc, x_d, t_d, wt_d, bt_d, wq_d, wk_d, wv_d, wp_d, bp_d, out_d)
    nc.compile()
    _CACHE["nc"] = nc
    return nc


def _run(inputs, trace=False, tmpdir=None):
    nc = _get_program()
    x = np.ascontiguousarray(np.asarray(inputs["x"], dtype=np.float32))
    t = np.ascontiguousarray(np.asarray(inputs["t"], dtype=np.float32))
    rep = {
        k: np.ascontiguousarray(np.asarray(inputs[k], dtype=np.float32))
        for k in ("W_t", "b_t", "Wq", "Wk", "Wv", "Wp", "bp")
    }
    in_maps = []
    for i in range(N_CORES):
        m = {"x": x[i * B:(i + 1) * B], "t": t[i * B:(i + 1) * B]}
        m.update(rep)
        in_maps.append(m)
    res = run_bass_kernel_spmd(nc, in_maps, list(range(N_CORES)),
                               trace=trace, tmpdir=tmpdir)
    out = np.concatenate([res.results[i]["out"] for i in range(N_CORES)], axis=0)
    return out, res


def kernel(**inputs):
    out, _ = _run(inputs)
    return out

